# revision 18
# baseline (speedup 1.0000x reference)
"""Trainium2 Bass/Tile kernel for nn_Attention_3418793967804.

8-way data parallel over batch (1 batch per NeuronCore). Per core:
qkv 1x1 conv (+folded BN), 4-head attention over 2304 positions,
depthwise 3x3 conv on v, residual add, final 1x1 conv (+folded BN).

v5: ACT-bound design. QK scores baseline-shaped (bf16 row-packed, two
[128, 1024] PSUM tiles per (ic=512, jb), one full bank per head — HW
requires one matmul output region per PSUM bank at offset 0). One exp
per (ic, jb, head-pair) on ACT writes fp8e4 p2 into a retained 3-pair
wave tile. Attention-value matmuls run in fp8 DoubleRow mode (256-deep
contraction per jb pair, 0.5 cycles/row, full 128-col stationaries with
a GV-ones denominator column at col 64 and zero padding); each (head,
i-subchunk) group accumulates 3 pairs in a rotating U bank, then folds
into an SBUF accumulator on the DVE across waves. Normalization:
accurate reciprocal of the SBUF denominator row, f32r PE row-broadcast,
tensor_mul; odd heads stage at partitions 0-63 and DMA into y rows
64-127. Convs stay f32r on the PE, riding the psS rotation; late
depthwise chunks are precomputed so only adds and the last c2 chunks
trail the final exp.
"""
import numpy as np

import concourse.bass as bass
import concourse.mybir as mybir
import concourse.tile as tile
from concourse import bacc

F32 = mybir.dt.float32
F32R = mybir.dt.float32r
BF16 = mybir.dt.bfloat16
F8 = mybir.dt.float8e4
EXP = mybir.ActivationFunctionType.Exp
DR = mybir.MatmulPerfMode.DoubleRow

CH = 256
HW = 2304
H = W = 48
NH = 4
DK = 32
DH = 64
SCALE = float(DK) ** -0.5
EPS = 1e-3
GV = 8.0         # power-of-2 gain on v for fp8; cancels exactly in y = U/den

IC_SIZES = [512, 512, 512, 512, 256]
IC_STARTS = [0, 512, 1024, 1536, 2048]
JB = 18          # 2304 / 128 j-blocks
NP = JB // 2     # 9 jb pairs (DoubleRow contraction)
QN = 384         # qkv/dwconv spatial chunk = 8 rows of 48
NQ = HW // QN    # 6
PW = 50          # padded width/height


def build_consts(qkv_w, qkv_g, qkv_b, qkv_m, qkv_v, c1_w, c1_g, c1_b, c1_m, c1_v,
                 c2_w, c2_g, c2_b, c2_m, c2_v):
    """Fold BN into conv weights and pack into device-layout numpy arrays."""
    import ml_dtypes
    f = np.float32
    sq = qkv_g / np.sqrt(qkv_v + EPS)
    Wq = (qkv_w[:, :, 0, 0] * sq[:, None]).astype(f)       # (512, 256)
    bq = (qkv_b - qkv_m * sq).astype(f)                    # (512,)
    s1 = c1_g / np.sqrt(c1_v + EPS)
    W1 = (c1_w[:, 0, :, :] * s1[:, None, None]).astype(f)  # (256, 3, 3)
    b1 = (c1_b - c1_m * s1).astype(f)
    s2 = c2_g / np.sqrt(c2_v + EPS)
    W2 = (c2_w[:, :, 0, 0] * s2[:, None]).astype(f)        # (256, 256)
    b2 = (c2_b - c2_m * s2).astype(f)

    # qkv output channel permutation: cols 0-127 Q_all (h*32+dk), 128-255 K_all,
    # 256-511 v in natural c = h*64+d order
    perm = np.zeros(512, dtype=np.int64)
    for col in range(128):
        h, dk = col // 32, col % 32
        perm[col] = 128 * h + dk
        perm[128 + col] = 128 * h + 32 + dk
    for col in range(256):
        h, d = col // 64, col % 64
        perm[256 + col] = 128 * h + 64 + d
    wt = np.ascontiguousarray(Wq[perm].T)                  # (256 ic, 512 col)
    bqkv = np.zeros((128, 4), f)
    for occ in range(4):
        bqkv[:, occ] = bq[perm[occ * 128:(occ + 1) * 128]]

    # depthwise conv diagonal stationaries: block (chunk, tap) at cols
    # (chunk*9+tap)*128, diag entries W1[chunk*128 + c, tap]
    diag = np.zeros((128, 18 * 128), f)
    for chunk in range(2):
        for tap in range(9):
            di, dj = tap // 3, tap % 3
            base = (chunk * 9 + tap) * 128
            idx = np.arange(128)
            diag[idx, base + idx] = W1[chunk * 128 + idx, di, dj]

    id8 = np.eye(128, dtype=np.float32).astype(ml_dtypes.float8_e4m3)
    w2t = np.ascontiguousarray(W2.T)                        # (256 c, 256 oc)
    # the dw bias passes linearly through the final conv: fold it there
    b2e = (b2 + W2 @ b1).astype(f)
    b2p = np.stack([b2e[0:128], b2e[128:256]], axis=1).astype(f)  # (128, 2)
    return dict(wt=wt, bqkv=bqkv, diag=diag, w2t=w2t, id8=id8, b2p=b2p)


def build_nc(debug=False):
    nc = bacc.Bacc("TRN2", target_bir_lowering=False, debug=False,
                   enable_asserts=True, num_devices=8)
    dp = {}
    def din(name, shape, dt=F32):
        dp[name] = nc.dram_tensor(name, list(shape), dt, kind="ExternalInput").ap()
    din("x", (256, HW), F32R)
    din("wt", (256, 512), F32R)
    din("bqkv", (128, 4))
    din("diag", (128, 18 * 128), F32R)
    din("w2t", (256, 256), F32R)
    din("b2p", (128, 2))
    din("id8", (128, 128), F8)
    out_d = nc.dram_tensor("out", [256, HW], F32, kind="ExternalOutput").ap()
    dbg = {}
    if debug:
        for name, shape in [("dq", (128, HW // 2)), ("dk", (128, HW // 2)),
                            ("dvt", (128, NP * 256)), ("dusb", (128, 2048)),
                            ("dy0", (128, HW)), ("dy1", (128, HW)),
                            ("dot0", (128, HW)), ("dot1", (128, HW))]:
            dbg[name] = nc.dram_tensor(name, list(shape), F32, kind="ExternalOutput").ap()

    with tile.TileContext(nc) as tc:
        build_body(nc, tc, dp, out_d, dbg)
    nc.compile()
    return nc


def build_body(nc, tc, dp, out_d, dbg):
    from contextlib import ExitStack
    with ExitStack() as ctx:
        ep = ctx.enter_context
        wpool = ep(tc.tile_pool(name="w", bufs=1))
        xpool = ep(tc.tile_pool(name="x", bufs=1))
        qkpool = ep(tc.tile_pool(name="qk", bufs=1))
        vtpool = ep(tc.tile_pool(name="vt", bufs=1))
        vppool = ep(tc.tile_pool(name="vp", bufs=1))
        wvpool = ep(tc.tile_pool(name="wv", bufs=2))
        uspool = ep(tc.tile_pool(name="us", bufs=2))
        ypool = ep(tc.tile_pool(name="y", bufs=1))
        npool = ep(tc.tile_pool(name="np", bufs=3))
        otpool = ep(tc.tile_pool(name="ot", bufs=1))
        obpool = ep(tc.tile_pool(name="ob", bufs=3))
        dwpool = ep(tc.tile_pool(name="dw", bufs=1))

        # --- weights & inputs ---
        wt_r = [wpool.tile([128, 512], F32R, tag=f"wt{c}", name=f"wt{c}") for c in range(2)]
        diag_r = wpool.tile([128, 18 * 128], F32R, tag="diag", name="diag")
        w2t_r = [wpool.tile([128, 256], F32R, tag=f"w2t{c}", name=f"w2t{c}") for c in range(2)]
        bq_f = wpool.tile([128, 4], F32, tag="bqf", name="bqf")
        b2_f = wpool.tile([128, 2], F32, tag="b2f", name="b2f")
        x_r = [xpool.tile([128, HW], F32R, tag=f"x{c}", name=f"x{c}") for c in range(2)]

        nc.sync.dma_start(bq_f[:], dp["bqkv"][:])
        nc.sync.dma_start(b2_f[:], dp["b2p"][:])
        for c in range(2):
            for qr in range(4):
                qsl = slice(qr * (HW // 4), (qr + 1) * (HW // 4))
                nc.sync.dma_start(x_r[c][:, qsl], dp["x"][128 * c:128 * (c + 1), qsl])
        for c in range(2):
            nc.sync.dma_start(wt_r[c][:], dp["wt"][128 * c:128 * (c + 1), :])

        Q = qkpool.tile([128, HW], BF16, tag="Q", name="Q")
        K = qkpool.tile([128, HW], BF16, tag="K", name="K")
        # VT8: per jb-pair p, 1024-col block [ktile(2) x (head(4) x 128)];
        # per head: cols 0-63 = V^T * GV, col 64 = GV ones (denominator fold),
        # cols 65-127 zero pad (DoubleRow ldweights need full 128-col tiles)
        VT8 = vtpool.tile([128, NP * 1024], F8, tag="VT8", name="VT8")
        nc.gpsimd.memset(VT8[:], 0.0)
        nc.gpsimd.memset(
            VT8[:].rearrange("q (a x) -> q a x", x=128)[:, :, 64:65], GV)
        onesb = vtpool.tile([128, 64], F32R, tag="onesb", name="onesb")
        nc.gpsimd.memset(onesb[:].bitcast(F32), 1.0)
        id_8 = vtpool.tile([128, 128], F8, tag="id8", name="id8")
        nc.sync.dma_start(id_8[:], dp["id8"][:])
        vp = [vppool.tile([128, PW * PW], F32R, tag=f"vp{c}", name=f"vp{c}") for c in range(2)]
        vf8 = [vppool.tile([128, HW], F8, tag=f"vf8{c}", name=f"vf8{c}") for c in range(2)]
        for c in range(2):
            nc.gpsimd.memset(vp[c][:].bitcast(F32), 0.0)
        y_all = [ypool.tile([128, HW], F32, tag=f"y{c}", name=f"y{c}") for c in range(2)]
        ot = [otpool.tile([128, HW], F32R, tag=f"ot{c}", name=f"ot{c}") for c in range(2)]
        dwsb = [dwpool.tile([128, 768], F32, tag=f"dwsb{c}", name=f"dwsb{c}") for c in range(2)]

        with tc.tile_pool(name="psS", bufs=3, space="PSUM") as psS, \
             tc.tile_pool(name="psU", bufs=2, space="PSUM") as psU:

            def emit_qkv(occ, g):
                # one 384-wide chunk of the qkv projection for output group occ
                ps = psS.tile([128, 1024], F32, tag="s2", name=f"qkv{occ}_{g}")
                sl = slice(g * QN, (g + 1) * QN)
                for c in range(2):
                    nc.tensor.matmul(
                        ps[:, 0:QN], wt_r[c][:, occ * 128:(occ + 1) * 128],
                        x_r[c][:, sl], start=(c == 0), stop=(c == 1))
                bias_ap = bq_f[:, occ:occ + 1]
                if occ == 0:
                    nc.vector.tensor_scalar_add(Q[:, sl], ps[:, 0:QN], bias_ap)
                elif occ == 1:
                    nc.vector.tensor_scalar_add(K[:, sl], ps[:, 0:QN], bias_ap)
                else:
                    c = occ - 2
                    vp3 = vp[c][:].rearrange("p (r w) -> p r w", w=PW)
                    dst = vp3[:, 1 + 8 * g:1 + 8 * g + 8, 1:49]
                    srcp = ps[:, 0:QN].rearrange("p (r w) -> p r w", w=48)
                    nc.vector.tensor_scalar_add(dst, srcp, bias_ap)
                    nc.vector.tensor_scalar(
                        vf8[c][:, sl], ps[:, 0:QN], bias_ap, GV,
                        mybir.AluOpType.add, mybir.AluOpType.mult)

            def emit_vtT(jb):
                # V^T 128x128 block transposes on the PE (fp8, via identity);
                # fp8 transpose writes with element step 2 (even slots), the
                # per-head copies compact into the padded VT8 layout.
                ps = psS.tile([128, 1024], F32, tag="s2", name=f"vt{jb}")
                p, t = jb // 2, jb % 2
                base = p * 1024 + t * 512
                for c in range(2):
                    outv = ps[:, 64 * c:64 * c + 64].bitcast(F8).rearrange(
                        "p (a two) -> p a two", two=2)[:, :, 0:1]
                    nc.tensor.matmul(outv,
                                     vf8[c][:, jb * 128:(jb + 1) * 128], id_8[:],
                                     is_transpose=True,
                                     start=(c == 0), stop=(c == 1))
                for c in range(2):
                    inv = ps[:, 64 * c:64 * c + 64].bitcast(F8).rearrange(
                        "p (a two) -> p a two", two=2)
                    for hh in range(2):
                        h = 2 * c + hh
                        nc.vector.tensor_copy(
                            VT8[:, base + 128 * h:base + 128 * h + 64],
                            inv[:, 64 * hh:64 * hh + 64, 0:1])

            def emit_dw_conv(c, g, store=None):
                # depthwise 3x3 conv chunk via 9 diagonal matmuls
                ps = psS.tile([128, 1024], F32, tag="s2", name=f"dw{c}_{g}")
                vp3 = vp[c][:].rearrange("p (r w) -> p r w", w=PW)
                for tap in range(9):
                    di, dj = tap // 3, tap % 3
                    mov = vp3[:, 8 * g + di:8 * g + di + 8, dj:dj + 48]
                    nc.tensor.matmul(
                        ps[:, 0:QN], diag_r[:, (c * 9 + tap) * 128:(c * 9 + tap + 1) * 128],
                        mov, start=(tap == 0), stop=(tap == 8))
                sl = slice(g * QN, (g + 1) * QN)
                if store is None:
                    nc.vector.tensor_add(ot[c][:, sl], ps[:, 0:QN], y_all[c][:, sl])
                else:
                    nc.vector.tensor_copy(dwsb[c][:, store * QN:(store + 1) * QN],
                                          ps[:, 0:QN])

            def emit_dw_add(c, g, store):
                sl = slice(g * QN, (g + 1) * QN)
                nc.vector.tensor_add(ot[c][:, sl],
                                     dwsb[c][:, store * QN:(store + 1) * QN],
                                     y_all[c][:, sl])

            def emit_c2(occ, j):
                # final 1x1 conv chunk over ot cols [384j, 384j+384)
                sl = slice(j * QN, (j + 1) * QN)
                ps = psS.tile([128, 1024], F32, tag="s2", name=f"c2_{occ}_{j}")
                for c in range(2):
                    nc.tensor.matmul(ps[:, 0:QN],
                                     w2t_r[c][:, occ * 128:(occ + 1) * 128],
                                     ot[c][:, sl], start=(c == 0), stop=(c == 1))
                ob = obpool.tile([128, 512], F32, tag="ob", name=f"ob{occ}_{j}")
                nc.vector.tensor_scalar_add(ob[:, 0:QN], ps[:, 0:QN],
                                            b2_f[:, occ:occ + 1])
                nc.sync.dma_start(out_d[occ * 128:(occ + 1) * 128, sl], ob[:, 0:QN])

            # qkv chunk schedule inside ic0's jb loop (K chunk g before jb 3g,
            # V chunk g before the transposes of jb 3g; Q g0/g1 up front)
            qkv_pre = [(0, 0), (0, 1), (1, 0), (2, 0), (3, 0)]
            qkv_sched = {
                0: [(1, 1)], 1: [(2, 1), (3, 1)], 2: [(0, 2)],
                3: [(1, 2)], 4: [(2, 2), (3, 2)], 5: [(0, 3)],
                6: [(1, 3)], 7: [(2, 3), (3, 3)], 8: [(0, 4)],
                9: [(1, 4)], 10: [(2, 4), (3, 4)], 11: [(0, 5)],
                12: [(1, 5)], 13: [(2, 5), (3, 5)],
            }
            for occ, g in qkv_pre:
                emit_qkv(occ, g)

            # ic-boundary conv schedule ('dwc' conv+add, 'dws' conv+stash,
            # 'dwa' stashed add, 'c2' final conv chunk)
            trans_sched = {
                0: [('dwc', 0, 0, None), ('dwc', 1, 0, None)],
                1: [('dwc', 0, 1, None), ('dwc', 1, 1, None),
                    ('c2', 0, 0, None), ('c2', 1, 0, None)],
                2: [('dwc', 0, 2, None), ('dwc', 1, 2, None),
                    ('dwc', 0, 3, None), ('dwc', 1, 3, None),
                    ('c2', 0, 1, None), ('c2', 1, 1, None),
                    ('c2', 0, 2, None), ('c2', 1, 2, None),
                    ('dws', 0, 4, 0), ('dws', 1, 4, 0),
                    ('dws', 0, 5, 1), ('dws', 1, 5, 1)],
                3: [('dwa', 0, 4, 0), ('dwa', 1, 4, 0),
                    ('c2', 0, 3, None), ('c2', 1, 3, None)],
                4: [('dwa', 0, 5, 1), ('dwa', 1, 5, 1),
                    ('c2', 0, 4, None), ('c2', 1, 4, None),
                    ('c2', 0, 5, None), ('c2', 1, 5, None)],
            }

            Usb_dbg = [None]
            for ic in range(5):
                n = IC_SIZES[ic]
                i0 = IC_STARTS[ic]
                isl = slice(i0, i0 + n)
                ncg = n // 256
                # SBUF accumulator: (head, subchunk) at cols (2h+cg)*256;
                # row 64 carries the folded denominator
                Usb = uspool.tile([128, 2048], F32, tag="Usb", name=f"Usb{ic}")
                Usb_dbg[0] = Usb
                wv = [None]

                def emit_qk_exp(jb):
                    jsl = slice(jb * 128, (jb + 1) * 128)
                    pw, t = (jb // 2) % 3, jb % 2
                    if pw == 0 and t == 0:
                        wv[0] = wvpool.tile([128, 3 * 4096], F8, tag="wv", name="wv")
                    for hp in range(2):
                        s2 = psS.tile([128, 1024], F32, tag="s2", name="s2")
                        for hh in range(2):
                            h = 2 * hp + hh
                            nc.tensor.matmul(
                                s2[:, 512 * hh:512 * hh + n],
                                K[32 * h:32 * (h + 1), jsl],
                                Q[32 * h:32 * (h + 1), isl],
                                start=True, stop=True, tile_position=(32 * h, 0))
                        base = pw * 4096 + t * 2048 + hp * 1024
                        if n == 512:
                            nc.scalar.activation(wv[0][:, base:base + 1024],
                                                 s2[:], EXP, scale=SCALE)
                        else:
                            dst = wv[0][:, base:base + 1024].rearrange(
                                "q (hh y) -> q hh y", hh=2)[:, :, 0:n]
                            src = s2[:].rearrange("q (hh y) -> q hh y", hh=2)[:, :, 0:n]
                            nc.scalar.activation(dst, src, EXP, scale=SCALE)
                    return wv[0]

                def emit_av_wave(w, wvt):
                    # fp8 DoubleRow AV: per (head, 256-col i-subchunk) group,
                    # 3 jb-pair matmuls accumulate in a rotating U bank, then
                    # fold into the SBUF accumulator on the DVE.
                    for h in range(4):
                        for cg in range(ncg):
                            U = psU.tile([128, 512], F32, tag="U",
                                         name=f"U{ic}_{w}_{h}_{cg}")
                            mb = (h // 2) * 1024 + (h % 2) * 512 + cg * 256
                            for pw in range(3):
                                p = 3 * w + pw
                                mov = wvt[:].rearrange(
                                    "q (pw t x) -> q pw t x", pw=3, t=2)[
                                    :, pw, :, mb:mb + 256]
                                vst = VT8[:, p * 1024:(p + 1) * 1024].rearrange(
                                    "q (t x) -> q t x", t=2)[:, :, 128 * h:128 * h + 128]
                                nc.tensor.matmul(
                                    U[0:128, 0:256], vst, mov,
                                    start=(pw == 0), stop=(pw == 2),
                                    perf_mode=DR, tile_position=(0, 0))
                            usl = slice((2 * h + cg) * 256, (2 * h + cg) * 256 + 256)
                            if w == 0:
                                nc.vector.tensor_copy(Usb[0:65, usl], U[0:65, 0:256])
                            else:
                                nc.vector.scalar_tensor_tensor(
                                    Usb[0:65, usl], U[0:65, 0:256], 1.0,
                                    Usb[0:65, usl],
                                    mybir.AluOpType.mult, mybir.AluOpType.add)

                for jb in range(JB):
                    if ic == 0:
                        for occ, g in qkv_sched.get(jb, ()):
                            emit_qkv(occ, g)
                    wvt = emit_qk_exp(jb)
                    if ic == 0:
                        emit_vtT(jb)
                    if jb % 6 == 5:
                        emit_av_wave(jb // 6, wvt)
                    if (ic, jb) == (0, 5):
                        for c in range(2):
                            nc.sync.dma_start(w2t_r[c][:], dp["w2t"][128 * c:128 * (c + 1), :])
                        nc.sync.dma_start(diag_r[:], dp["diag"][:])

                # normalization: y = U * (1/den); reciprocal of Usb row 64,
                # f32r PE broadcast to rows 0-63, multiply. Odd heads stage at
                # rows 0-63 and DMA into y rows 64-127 (partition move).
                rden = npool.tile([128, 2048], F32R, tag="rden", name=f"rden{ic}")
                with nc.allow_low_precision(reason="f32r recip feeds exact-1.0 bcast"):
                    for h in range(4):
                        for cg in range(ncg):
                            usl = slice((2 * h + cg) * 256, (2 * h + cg) * 256 + 256)
                            nc.vector.reciprocal(rden[64:65, usl], Usb[64:65, usl])
                ystg = npool.tile([64, 1024], F32, tag="ystg", name=f"ystg{ic}")
                for hp in range(2):
                    for cg in range(ncg):
                        # R tile per (head pair, subchunk): even head bank 1
                        # (cols 0:256), odd head bank 2 (cols 512:768)
                        R = psS.tile([128, 1024], F32, tag="s2",
                                     name=f"R{ic}_{hp}_{cg}")
                        for hh in range(2):
                            h = 2 * hp + hh
                            usl = slice((2 * h + cg) * 256, (2 * h + cg) * 256 + 256)
                            nc.tensor.matmul(
                                R[0:64, 512 * hh:512 * hh + 256],
                                onesb[64:65, :], rden[64:65, usl],
                                start=True, stop=True, tile_position=(64, 0))
                        r2c = npool.tile([64, 512], F32, tag="r2c",
                                         name=f"r2c{ic}_{hp}_{cg}")
                        nc.vector.tensor_copy(
                            r2c[:].rearrange("q (a x) -> q a x", x=256),
                            R[0:64, :].rearrange("q (a x) -> q a x", x=512)[:, :, 0:256])
                        ysl = slice(i0 + cg * 256, i0 + cg * 256 + 256)
                        ue = slice((2 * (2 * hp) + cg) * 256,
                                   (2 * (2 * hp) + cg) * 256 + 256)
                        uo = slice((2 * (2 * hp + 1) + cg) * 256,
                                   (2 * (2 * hp + 1) + cg) * 256 + 256)
                        nc.vector.tensor_mul(y_all[hp][0:64, ysl],
                                             Usb[0:64, ue], r2c[:, 0:256])
                        nc.vector.tensor_mul(
                            ystg[0:64, (2 * hp + cg) * 256:(2 * hp + cg) * 256 + 256],
                            Usb[0:64, uo], r2c[:, 256:512])
                for hp in range(2):
                    for cg in range(ncg):
                        ysl = slice(i0 + cg * 256, i0 + cg * 256 + 256)
                        nc.sync.dma_start(
                            y_all[hp][64:128, ysl],
                            ystg[0:64, (2 * hp + cg) * 256:(2 * hp + cg) * 256 + 256])

                for kind, a, b_, st in trans_sched.get(ic, []):
                    if kind == 'dwc':
                        emit_dw_conv(a, b_)
                    elif kind == 'dws':
                        emit_dw_conv(a, b_, store=st)
                    elif kind == 'dwa':
                        emit_dw_add(a, b_, st)
                    else:
                        emit_c2(a, b_)

        if dbg:
            nc.sync.dma_start(dbg["dusb"][:], Usb_dbg[0][:])
            nc.sync.dma_start(dbg["dq"][:], Q[:].bitcast(F32))
            nc.sync.dma_start(dbg["dk"][:], K[:].bitcast(F32))
            nc.sync.dma_start(dbg["dvt"][:], VT8[:].bitcast(F32))
            nc.sync.dma_start(dbg["dy0"][:], y_all[0][:])
            nc.sync.dma_start(dbg["dy1"][:], y_all[1][:])
            nc.sync.dma_start(dbg["dot0"][:], ot[0][:].bitcast(F32))
            nc.sync.dma_start(dbg["dot1"][:], ot[1][:].bitcast(F32))


def make_in_maps(x_full, consts):
    maps = []
    for b in range(8):
        m = dict(consts)
        m["x"] = np.ascontiguousarray(x_full[b].reshape(256, HW), dtype=np.float32)
        maps.append(m)
    return maps

_CACHED = {}


def _get_nc():
    if 'nc' not in _CACHED:
        _CACHED['nc'] = build_nc(debug=False)
    return _CACHED['nc']


def kernel(**inputs):
    """Full (unsharded) inputs -> full output (8, 256, 48, 48) float32."""
    from concourse.bass_utils import run_bass_kernel_spmd

    x = np.asarray(inputs['x'], dtype=np.float32)
    consts = build_consts(**{k: np.asarray(v) for k, v in inputs.items()
                             if k != 'x'})
    in_maps = make_in_maps(x, consts)
    nc = _get_nc()
    res = run_bass_kernel_spmd(nc, in_maps, list(range(8)))
    out = np.stack([res.results[b]['out'].reshape(256, 48, 48)
                    for b in range(8)])
    return out.astype(np.float32)
